# revision 5
# baseline (speedup 1.0000x reference)
"""Single-head causal attention (B=8, T=2048, D=1024, H=128) on 8 TRN2 cores.

Data-parallel over batch: core c computes batch element c entirely on-chip.
All matmuls run as float32r (FP22 multiply / fp32 accumulate).

Dataflow per core (all SPMD, no collectives):
  x [T, D] --PE transpose--> xT [D, T] (SBUF)
  qT/kT/vT [H, T] = W.T @ xT  (W chunks stationary, N=512 moving)
  v [T, H] = PE transpose of vT
  per 512-wide q superblock:
    scores^T[k_blk, q] = kT_blk.T @ qT_sb   (psum)
    (+ causal mask on diagonal blocks) -> exp on ACT -> probs^T (SBUF)
    out^T  += v_blk.T @ probs^T  ;  l += ones.T @ probs^T   (psum)
    out^T / l -> PE transpose -> DMA out
"""

import os
import sys

if "/opt/trn_rl_repo" not in sys.path:
    sys.path.insert(0, "/opt/trn_rl_repo")

import numpy as np

import concourse.bass as bass
import concourse.mybir as mybir
import concourse.tile as tile
from concourse import bacc
from concourse.bass import ts
from concourse.bass_interp import get_hw_module
from concourse.bass_utils import run_bass_kernel_spmd
from concourse.masks import make_identity

T = 2048
D = 1024
H = 128
B = 8
P = 128
QB = 512                 # q superblock width
N_SB = T // QB           # 4 superblocks
N_TB = T // P            # 16 token blocks
N_DC = D // P            # 8 embed chunks
SCALE = D ** -0.5
NEG = -1e30

F32 = mybir.dt.float32
F32R = mybir.dt.float32r
EXP = mybir.ActivationFunctionType.Exp


def build_attention(nc, tc, x, wq, wk, wv, out):
    """Emit the attention program for one core into TileContext tc."""
    from contextlib import ExitStack

    ctx = ExitStack()
    with ctx:
        _build_attention(ctx, nc, tc, x, wq, wk, wv, out)


def _build_attention(ctx, nc, tc, x, wq, wk, wv, out):
    singles = ctx.enter_context(tc.tile_pool(name="singles", bufs=1))

    # --- constants ---
    ident = singles.tile([P, P], F32, name="ident")
    make_identity(nc, ident[:])
    ones_f32 = singles.tile([P, P], F32, name="ones_f32")
    nc.vector.memset(ones_f32[:], 1.0)
    ones_sq = singles.tile([P, P], F32R, name="ones_sq")
    nc.vector.tensor_copy(out=ones_sq[:], in_=ones_f32[:])

    # additive causal masks for the 4 diagonal sub-positions:
    # mask_dj[p, f] = 0 where f >= p + 128*dj else NEG
    masks = []
    for dj in range(4):
        m = singles.tile([P, QB], F32, name=f"mask_{dj}")
        nc.gpsimd.memset(m[:], 0.0)
        nc.gpsimd.affine_select(
            out=m[:], in_=m[:],
            compare_op=mybir.AluOpType.is_ge,
            fill=NEG,
            base=-P * dj,
            channel_multiplier=-1,
            pattern=[[1, QB]],
        )
        masks.append(m)

    # --- weights: [D, H] -> [P, N_DC, H] (d_inner, d_chunk, h) ---
    w_sb = []
    for wap, nm in ((wq, "wq"), (wk, "wk"), (wv, "wv")):
        wt = singles.tile([P, N_DC, H], F32R, name=f"{nm}_sb")
        nc.sync.dma_start(out=wt[:], in_=wap.rearrange("(o p) h -> p o h", p=P).bitcast(F32R))
        w_sb.append(wt)

    # --- persistent activations ---
    xTs = [singles.tile([P, T], F32R, name=f"xT_{d}") for d in range(N_DC)]
    qT = singles.tile([P, T], F32R, name="qT")
    kT = singles.tile([P, T], F32R, name="kT")
    vT = singles.tile([P, T], F32, name="vT")
    v_sb = singles.tile([P, T], F32R, name="v_sb")  # [t_inner, t_blk*H]

    ps_rot = ctx.enter_context(tc.tile_pool(name="ps_rot", bufs=3, space="PSUM"))
    ps_acc = ctx.enter_context(tc.tile_pool(name="ps_acc", bufs=4, space="PSUM"))
    tmp = ctx.enter_context(tc.tile_pool(name="tmp", bufs=2))
    xnat = ctx.enter_context(tc.tile_pool(name="xnat", bufs=6))
    probs_pool = ctx.enter_context(tc.tile_pool(name="probs", bufs=17))

    # --- phase 1: load x and transpose to xT ---
    for gg in range(4):  # groups of 4 token blocks
        x4 = []
        for g4 in range(4):
            g = gg * 4 + g4
            xt = xnat.tile([P, D], F32, tag="xnat", name=f"xnat_{g}")
            nc.sync.dma_start(out=xt[:], in_=x[g * P:(g + 1) * P, :])
            x4.append(xt)
        for dc in range(N_DC):
            ps = ps_rot.tile([P, QB], F32, tag="rot", name=f"ps_tr_{gg}_{dc}")
            for g4 in range(4):
                nc.tensor.transpose(ps[:, ts(g4, P)], x4[g4][:, ts(dc, P)], ident[:])
            if dc % 2 == 0:
                nc.vector.tensor_copy(out=xTs[dc][:, ts(gg, QB)], in_=ps[:])
            else:
                nc.scalar.copy(out=xTs[dc][:, ts(gg, QB)], in_=ps[:])

    # --- phase 2: QKV^T projections (fold softmax scale into qT) ---
    for w_i, (wt, dst) in enumerate(((w_sb[0], qT), (w_sb[1], kT), (w_sb[2], vT))):
        for c in range(N_SB):
            ps = ps_rot.tile([P, QB], F32, tag="rot", name=f"ps_p_{w_i}_{c}")
            for dc in range(N_DC):
                nc.tensor.matmul(
                    ps[:],
                    lhsT=wt[:, dc, :],
                    rhs=xTs[dc][:, ts(c, QB)],
                    start=(dc == 0),
                    stop=(dc == N_DC - 1),
                )
            if w_i == 0:
                nc.scalar.mul(out=dst[:, ts(c, QB)], in_=ps[:], mul=SCALE)
            elif w_i == 1:
                nc.vector.tensor_copy(out=dst[:, ts(c, QB)], in_=ps[:])
            else:
                nc.scalar.copy(out=dst[:, ts(c, QB)], in_=ps[:])

    # --- phase 3: v = transpose(vT) ---
    for gg in range(4):
        ps = ps_rot.tile([P, QB], F32, tag="rot", name=f"ps_v_{gg}")
        for g4 in range(4):
            g = gg * 4 + g4
            nc.tensor.transpose(ps[:, ts(g4, P)], vT[:, ts(g, P)], ident[:])
        nc.vector.tensor_copy(out=v_sb[:, ts(gg, QB)], in_=ps[:])

    # --- phase 4: attention per q superblock ---
    for I in range(N_SB):
        nj = 4 * I + 4
        probs = []
        for j in range(nj):
            ps_s = ps_rot.tile([P, QB], F32, tag="rot", name=f"ps_s_{I}_{j}")
            nc.tensor.matmul(
                ps_s[:],
                lhsT=kT[:, ts(j, P)],
                rhs=qT[:, ts(I, QB)],
                start=True,
                stop=True,
            )
            dj = j - 4 * I
            if dj >= 0:
                nc.vector.tensor_add(ps_s[:], ps_s[:], masks[dj][:])
            pj = probs_pool.tile([P, QB], F32R, tag="probs", name=f"probs_{I}_{j}")
            nc.scalar.activation(pj[:], ps_s[:], EXP)
            probs.append(pj)

        ps_o = ps_acc.tile([P, QB], F32, tag="acc", name=f"ps_o_{I}")
        for j in range(nj):
            nc.tensor.matmul(
                ps_o[:],
                lhsT=v_sb[:, ts(j, P)],
                rhs=probs[j][:],
                start=(j == 0),
                stop=(j == nj - 1),
            )
        ps_l = ps_acc.tile([P, QB], F32, tag="acc", name=f"ps_l_{I}")
        for j in range(nj):
            nc.tensor.matmul(
                ps_l[:],
                lhsT=ones_sq[:],
                rhs=probs[j][:],
                start=(j == 0),
                stop=(j == nj - 1),
            )

        rec = tmp.tile([P, QB], F32, tag="rec", name=f"rec_{I}")
        nc.vector.reciprocal(rec[:], ps_l[:])
        otn = tmp.tile([P, QB], F32, tag="otn", name=f"otn_{I}")
        nc.vector.tensor_mul(otn[:], ps_o[:], rec[:])

        ps_t = ps_rot.tile([P, QB], F32, tag="rot", name=f"ps_t_{I}")
        for b4 in range(4):
            nc.tensor.transpose(ps_t[:, ts(b4, P)], otn[:, ts(b4, P)], ident[:])
        stage = tmp.tile([P, QB], F32, tag="stage", name=f"stage_{I}")
        nc.scalar.copy(out=stage[:], in_=ps_t[:])
        nc.sync.dma_start(
            out=out[I * QB:(I + 1) * QB, :].rearrange("(b p) h -> p b h", p=P),
            in_=stage[:].rearrange("p (b h) -> p b h", b=4),
        )


def build_nc(repeat: int = 1):
    """Build + compile the Bass program. repeat>1 wraps the body in a HW loop
    (for timing measurements only)."""
    nc = bacc.Bacc("TRN2", target_bir_lowering=False, debug=False, num_devices=B)
    x = nc.dram_tensor("x", [T, D], F32, kind="ExternalInput").ap()
    wq = nc.dram_tensor("Wq", [D, H], F32, kind="ExternalInput").ap()
    wk = nc.dram_tensor("Wk", [D, H], F32, kind="ExternalInput").ap()
    wv = nc.dram_tensor("Wv", [D, H], F32, kind="ExternalInput").ap()
    out = nc.dram_tensor("out", [T, H], F32, kind="ExternalOutput").ap()

    with tile.TileContext(nc) as tc:
        if repeat == 1:
            build_attention(nc, tc, x, wq, wk, wv, out)
        else:
            with tc.For_i(0, repeat, 1):
                build_attention(nc, tc, x, wq, wk, wv, out)

    nc.compile()
    return nc


_NC_CACHE = {}


def _get_nc(repeat: int = 1):
    if repeat not in _NC_CACHE:
        _NC_CACHE[repeat] = build_nc(repeat)
    return _NC_CACHE[repeat]


def run_on_hw(nc, in_maps):
    old = nc.m
    nc.m = get_hw_module(nc.m)
    try:
        return run_bass_kernel_spmd(nc, in_maps, list(range(B)))
    finally:
        nc.m = old


def kernel(x, Wq, Wk, Wv):
    x = np.ascontiguousarray(np.asarray(x, dtype=np.float32))
    Wq = np.ascontiguousarray(np.asarray(Wq, dtype=np.float32))
    Wk = np.ascontiguousarray(np.asarray(Wk, dtype=np.float32))
    Wv = np.ascontiguousarray(np.asarray(Wv, dtype=np.float32))
    assert x.shape == (B, T, D)

    nc = _get_nc(1)
    in_maps = [{"x": x[c], "Wq": Wq, "Wk": Wk, "Wv": Wv} for c in range(B)]
    res = run_on_hw(nc, in_maps)
    return np.stack([res.results[c]["out"] for c in range(B)], axis=0)


# revision 9
# speedup vs baseline: 3.8737x; 3.8737x over previous
"""Single-head causal attention (B=8, T=2048, D=1024, H=128) on 8 TRN2 cores.

Data-parallel over batch: core c computes batch element c entirely on-chip.
All matmuls run as float32r (FP22 multiply / fp32 accumulate).

Dataflow per core (all SPMD, no collectives):
  x [T, D] --PE transpose--> xT [D, T] (SBUF)
  qT/kT/vT [H, T] = W.T @ xT  (W chunks stationary, N=512 moving)
  v [T, H] = PE transpose of vT
  per 512-wide q superblock:
    scores^T[k_blk, q] = kT_blk.T @ qT_sb   (pairs of k blocks -> 2-bank psum)
    (+ causal mask on diagonal blocks) -> exp on ACT -> probs^T (SBUF)
    out^T  += v_blk.T @ probs^T  ;  l += ones.T @ probs^T   (psum)
    out^T / l -> PE transpose -> DMA out
"""

import os
import sys

if "/opt/trn_rl_repo" not in sys.path:
    sys.path.insert(0, "/opt/trn_rl_repo")

from contextlib import ExitStack

import numpy as np

import concourse.bass as bass
import concourse.mybir as mybir
import concourse.tile as tile
from concourse import bacc
from concourse.bass import ts
from concourse.bass_interp import get_hw_module
from concourse.bass_utils import run_bass_kernel_spmd
from concourse.masks import make_identity

T = 2048
D = 1024
H = 128
B = 8
P = 128
QB = 512                 # q superblock width
N_SB = T // QB           # 4 superblocks
N_TB = T // P            # 16 token blocks
N_DC = D // P            # 8 embed chunks
SCALE = D ** -0.5
NEG = -1e30

F32 = mybir.dt.float32
F32R = mybir.dt.float32r
EXP = mybir.ActivationFunctionType.Exp


def build_attention(nc, tc, x, wq, wk, wv, out):
    """Emit the attention program for one core into TileContext tc."""
    ctx = ExitStack()
    with ctx:
        _build_attention(ctx, nc, tc, x, wq, wk, wv, out)


def _build_attention(ctx, nc, tc, x, wq, wk, wv, out):
    singles = ctx.enter_context(tc.tile_pool(name="singles", bufs=1))

    # --- constants ---
    ident_f = singles.tile([P, P], F32, name="ident_f")
    make_identity(nc, ident_f[:])
    ident = singles.tile([P, P], F32R, name="ident")
    nc.vector.tensor_copy(out=ident[:], in_=ident_f[:])
    ones_f = singles.tile([P, P], F32, name="ones_f")
    nc.vector.memset(ones_f[:], 1.0)
    ones_sq = singles.tile([P, P], F32R, name="ones_sq")
    nc.vector.tensor_copy(out=ones_sq[:], in_=ones_f[:])

    # paired additive causal masks: pair m covers dj = 2m (cols 0:512) and
    # dj = 2m+1 (cols 512:1024); mask[p, f] = 0 where f >= p + 128*dj else NEG
    masks2 = []
    for m in range(2):
        mk = singles.tile([P, 2 * QB], F32, name=f"mask2_{m}")
        nc.gpsimd.memset(mk[:], 0.0)
        for half in range(2):
            dj = 2 * m + half
            nc.gpsimd.affine_select(
                out=mk[:, half * QB:(half + 1) * QB],
                in_=mk[:, half * QB:(half + 1) * QB],
                compare_op=mybir.AluOpType.is_ge,
                fill=NEG,
                base=-P * dj,
                channel_multiplier=-1,
                pattern=[[1, QB]],
            )
        masks2.append(mk)

    # --- weights: [D, H] -> [P, N_DC, H] (d_inner, d_chunk, h) ---
    w_sb = []
    for wap, nm in ((wq, "wq"), (wk, "wk"), (wv, "wv")):
        wt = singles.tile([P, N_DC, H], F32R, name=f"{nm}_sb")
        nc.sync.dma_start(
            out=wt[:], in_=wap.rearrange("(o p) h -> p o h", p=P).bitcast(F32R)
        )
        w_sb.append(wt)

    # --- persistent activations ---
    xTs = [singles.tile([P, T], F32R, name=f"xT_{d}") for d in range(N_DC)]
    qT = singles.tile([P, T], F32R, name="qT")
    kT = singles.tile([P, T], F32R, name="kT")
    vT = singles.tile([P, T], F32R, name="vT")
    v_sb = singles.tile([P, T], F32R, name="v_sb")  # [t_inner, t_blk*H]

    ps_rot = ctx.enter_context(tc.tile_pool(name="ps_rot", bufs=2, space="PSUM"))
    ps_s2 = ctx.enter_context(tc.tile_pool(name="ps_s2", bufs=2, space="PSUM"))
    ps_acc = ctx.enter_context(tc.tile_pool(name="ps_acc", bufs=2, space="PSUM"))
    tmp = ctx.enter_context(tc.tile_pool(name="tmp", bufs=2))
    xnat = ctx.enter_context(tc.tile_pool(name="xnat", bufs=4))
    probs_pool = ctx.enter_context(tc.tile_pool(name="probs", bufs=9))

    # --- phase 1: per 512-token chunk: load x, transpose, project, build v ---
    for gg in range(4):
        x4 = []
        for g4 in range(4):
            g = gg * 4 + g4
            xt = xnat.tile([P, D], F32R, tag="xnat", name=f"xnat_{g}")
            nc.sync.dma_start(out=xt[:], in_=x[g * P:(g + 1) * P, :].bitcast(F32R))
            x4.append(xt)
        for dc in range(N_DC):
            ps = ps_rot.tile([P, QB], F32R, tag="rot", name=f"ps_tr_{gg}_{dc}")
            for g4 in range(4):
                nc.tensor.transpose(ps[:, ts(g4, P)], x4[g4][:, ts(dc, P)], ident[:])
            if dc % 2 == 0:
                nc.vector.tensor_copy(out=xTs[dc][:, ts(gg, QB)], in_=ps[:].bitcast(F32))
            else:
                nc.scalar.copy(out=xTs[dc][:, ts(gg, QB)], in_=ps[:].bitcast(F32))

        # QKV^T projections for this 512-token chunk (scale folded into qT)
        for w_i, (wt, dst) in enumerate(((w_sb[0], qT), (w_sb[1], kT), (w_sb[2], vT))):
            ps = ps_rot.tile([P, QB], F32, tag="rot", name=f"ps_p_{w_i}_{gg}")
            for dc in range(N_DC):
                nc.tensor.matmul(
                    ps[:],
                    lhsT=wt[:, dc, :],
                    rhs=xTs[dc][:, ts(gg, QB)],
                    start=(dc == 0),
                    stop=(dc == N_DC - 1),
                )
            if w_i == 0:
                nc.scalar.mul(out=dst[:, ts(gg, QB)], in_=ps[:], mul=SCALE)
            else:
                nc.vector.tensor_copy(out=dst[:, ts(gg, QB)], in_=ps[:])

        # v chunk = transpose(vT chunk)
        psv = ps_rot.tile([P, QB], F32R, tag="rot", name=f"ps_v_{gg}")
        for g4 in range(4):
            g = gg * 4 + g4
            nc.tensor.transpose(psv[:, ts(g4, P)], vT[:, ts(g, P)], ident[:])
        nc.scalar.copy(out=v_sb[:, ts(gg, QB)], in_=psv[:].bitcast(F32))

    # --- phase 2: attention per q superblock, k blocks in pairs ---
    for I in range(N_SB):
        nj = 4 * I + 4
        npair = nj // 2
        probs = []  # [128, 1024] tiles, pair p covers k blocks (2p, 2p+1)
        for pr in range(npair):
            ps2 = ps_s2.tile([P, 2 * QB], F32, tag="s2", name=f"ps_s_{I}_{pr}")
            for half in range(2):
                j = 2 * pr + half
                nc.tensor.matmul(
                    ps2[:, ts(half, QB)],
                    lhsT=kT[:, ts(j, P)],
                    rhs=qT[:, ts(I, QB)],
                    start=True,
                    stop=True,
                )
            dpr = pr - 2 * I  # diagonal pair index (0 or 1 when on diagonal)
            if dpr >= 0:
                nc.vector.tensor_add(ps2[:], ps2[:], masks2[dpr][:])
            pj = probs_pool.tile([P, 2 * QB], F32R, tag="probs", name=f"probs_{I}_{pr}")
            nc.scalar.activation(pj[:], ps2[:], EXP)
            probs.append(pj)

        ps_o = ps_acc.tile([P, QB], F32, tag="acc", name=f"ps_o_{I}")
        for j in range(nj):
            nc.tensor.matmul(
                ps_o[:],
                lhsT=v_sb[:, ts(j, P)],
                rhs=probs[j // 2][:, ts(j % 2, QB)],
                start=(j == 0),
                stop=(j == nj - 1),
            )
        ps_l = ps_acc.tile([P, QB], F32, tag="acc", name=f"ps_l_{I}")
        for j in range(nj):
            nc.tensor.matmul(
                ps_l[:],
                lhsT=ones_sq[:],
                rhs=probs[j // 2][:, ts(j % 2, QB)],
                start=(j == 0),
                stop=(j == nj - 1),
            )

        rec = tmp.tile([P, QB], F32, tag="rec", name=f"rec_{I}")
        nc.vector.reciprocal(rec[:], ps_l[:])
        otn = tmp.tile([P, QB], F32R, tag="otn", name=f"otn_{I}")
        nc.vector.tensor_mul(otn[:], ps_o[:], rec[:])

        ps_t = ps_rot.tile([P, QB], F32R, tag="rot", name=f"ps_t_{I}")
        for b4 in range(4):
            nc.tensor.transpose(ps_t[:, ts(b4, P)], otn[:, ts(b4, P)], ident[:])
        stage = tmp.tile([P, QB], F32, tag="stage", name=f"stage_{I}")
        if I % 2 == 0:
            nc.scalar.copy(out=stage[:], in_=ps_t[:].bitcast(F32))
        else:
            nc.vector.tensor_copy(out=stage[:], in_=ps_t[:].bitcast(F32))
        nc.sync.dma_start(
            out=out[I * QB:(I + 1) * QB, :].rearrange("(b p) h -> p b h", p=P),
            in_=stage[:].rearrange("p (b h) -> p b h", b=4),
        )


def build_nc(repeat: int = 1):
    """Build + compile the Bass program. repeat>1 wraps the body in a HW loop
    (for timing measurements only)."""
    nc = bacc.Bacc("TRN2", target_bir_lowering=False, debug=False, num_devices=B)
    x = nc.dram_tensor("x", [T, D], F32, kind="ExternalInput").ap()
    wq = nc.dram_tensor("Wq", [D, H], F32, kind="ExternalInput").ap()
    wk = nc.dram_tensor("Wk", [D, H], F32, kind="ExternalInput").ap()
    wv = nc.dram_tensor("Wv", [D, H], F32, kind="ExternalInput").ap()
    out = nc.dram_tensor("out", [T, H], F32, kind="ExternalOutput").ap()

    with tile.TileContext(nc) as tc:
        if repeat == 1:
            build_attention(nc, tc, x, wq, wk, wv, out)
        else:
            with tc.For_i(0, repeat, 1):
                build_attention(nc, tc, x, wq, wk, wv, out)

    nc.compile()
    return nc


_NC_CACHE = {}


def _get_nc(repeat: int = 1):
    if repeat not in _NC_CACHE:
        _NC_CACHE[repeat] = build_nc(repeat)
    return _NC_CACHE[repeat]


def run_on_hw(nc, in_maps):
    old = nc.m
    nc.m = get_hw_module(nc.m)
    try:
        return run_bass_kernel_spmd(nc, in_maps, list(range(B)))
    finally:
        nc.m = old


def kernel(x, Wq, Wk, Wv):
    x = np.ascontiguousarray(np.asarray(x, dtype=np.float32))
    Wq = np.ascontiguousarray(np.asarray(Wq, dtype=np.float32))
    Wk = np.ascontiguousarray(np.asarray(Wk, dtype=np.float32))
    Wv = np.ascontiguousarray(np.asarray(Wv, dtype=np.float32))
    assert x.shape == (B, T, D)

    nc = _get_nc(1)
    in_maps = [{"x": x[c], "Wq": Wq, "Wk": Wk, "Wv": Wv} for c in range(B)]
    res = run_on_hw(nc, in_maps)
    return np.stack([res.results[c]["out"] for c in range(B)], axis=0)
